# revision 1
# baseline (speedup 1.0000x reference)
"""Trainium2 Bass kernel: 600-bin bincount of 33.5M int32 values in [0, 600).

Strategy (data-parallel over 8 NeuronCores, per the sharding hint):
  - shard x into 8 slices of 4,194,304 elements, one per core, laid out
    [128 partitions, 32768] in DRAM;
  - per core, stream [128, FD] chunks into SBUF and compute, per element,
    a bilinear feature factorization of the bin index (600 = 19*32):
      * moving side  (VectorE):  one-hot of l = x & 31   (32 fp16 features)
      * stationary side (ScalarE): ones + Sign(x - 32m + 0.5), m=1..18
        (cumulative >= threshold features of h = x >> 5, as +-1 signs)
  - TensorE contracts each 128-element group: one matmul per group
    accumulating the 19x32 joint count matrix into a single PSUM tile
    (fp32 accumulate is exact: all entries < 2^24);
  - host recovers the exact joint histogram from the sign algebra and
    sums the 8 per-core histograms.
"""

from contextlib import ExitStack

import numpy as np

import bass_rust
import concourse.bass as bass
import concourse.mybir as mybir
import concourse.tile as tile
from concourse.bass_utils import run_bass_kernel_spmd

N_TOTAL = 33554432
N_CORES = 8
P = 128
COLS = N_TOTAL // N_CORES // P  # 32768 elements per partition per core
FD = 512                        # chunk free-dim
CHUNKS = COLS // FD             # 64
NB_L = 32                       # moving one-hot width (l = x & 31)
STAT_W = 19                     # ones + 18 sign thresholds (h = x >> 5 in [0,19))
MINLENGTH = 600


def _split_excess_waits(nc, max_waits=1):
    """This walrus build accepts at most one semaphore wait per instruction
    on several instruction structs; hoist excess waits onto preceding
    same-engine Drains (engines execute in order, so a chain of single-wait
    drains is equivalent to one multi-wait instruction)."""
    for f in nc.m.functions:
        for bb in f.blocks:
            out = []
            changed = False
            for ins in bb.instructions:
                si = ins.sync_info
                if si is not None and len(si.on_wait) > max_waits:
                    waits = list(si.on_wait)
                    chunks = [
                        waits[j : j + max_waits]
                        for j in range(0, len(waits), max_waits)
                    ]
                    for ci, chunk in enumerate(chunks[:-1]):
                        pre = mybir.InstDrain(
                            name=f"{ins.name}-presplit{ci}", ins=[], outs=[]
                        )
                        pre.engine = ins.engine
                        pre.sync_info = bass_rust.SyncInfo(
                            on_wait=chunk, on_update=[]
                        )
                        out.append(pre)
                        changed = True
                    ins.sync_info = bass_rust.SyncInfo(
                        on_wait=chunks[-1], on_update=list(si.on_update)
                    )
                out.append(ins)
            if changed:
                bb.instructions = out


def _reg_const(nc, val):
    val = float(val)
    if (mybir.dt.float32, val) in nc.const_aps.aps:
        return
    t = nc.alloc_sbuf_tensor(
        f"constf32_{abs(val)}_{'n' if val < 0 else 'p'}", [128, 1], mybir.dt.float32
    )
    nc.gpsimd.memset(t.ap(), val)
    nc.const_aps.aps[(mybir.dt.float32, val)] = t.ap()


def build_kernel():
    nc = bass.Bass("TRN2", target_bir_lowering=False, debug=False)
    x = nc.dram_tensor("x", [P, COLS], mybir.dt.int32, kind="ExternalInput")
    y = nc.dram_tensor("y", [STAT_W, NB_L], mybir.dt.float32, kind="ExternalOutput")
    for m in range(1, STAT_W):
        _reg_const(nc, -(32.0 * m - 0.5))
    nc.all_engine_barrier()
    with tile.TileContext(nc) as tc:
        with tc.tile_pool(name="inp", bufs=2) as inp_pool, \
             tc.tile_pool(name="feat", bufs=2) as feat_pool, \
             tc.tile_pool(name="psum", bufs=1, space="PSUM") as psum_pool, \
             tc.tile_pool(name="outp", bufs=1) as out_pool:
            acc = psum_pool.tile([STAT_W, NB_L], mybir.dt.float32)
            for c in range(CHUNKS):
                xi = inp_pool.tile([P, FD], mybir.dt.int32, tag="xi")
                nc.gpsimd.dma_start(xi[:], x.ap()[:, c * FD:(c + 1) * FD])
                x16 = feat_pool.tile([P, FD], mybir.dt.float16, tag="x16")
                nc.vector.tensor_copy(x16[:], xi[:])
                l32 = feat_pool.tile([P, FD], mybir.dt.int32, tag="l32")
                nc.vector.tensor_scalar(
                    l32[:], xi[:], 31, None, mybir.AluOpType.bitwise_and
                )
                l16 = feat_pool.tile([P, FD], mybir.dt.float16, tag="l16")
                nc.vector.tensor_copy(l16[:], l32[:])
                mov = feat_pool.tile([P, NB_L, FD], mybir.dt.float16, tag="mov")
                for lb in range(NB_L):
                    nc.vector.tensor_scalar(
                        mov[:, lb, :], l16[:], float(lb), None,
                        mybir.AluOpType.is_equal,
                    )
                stat = feat_pool.tile([P, STAT_W, FD], mybir.dt.float16, tag="stat")
                nc.gpsimd.memset(stat[:, 0, :], 1.0)
                for m in range(1, STAT_W):
                    nc.scalar.activation(
                        stat[:, m, :], x16[:],
                        mybir.ActivationFunctionType.Sign,
                        bias=-(32.0 * m - 0.5), scale=1.0,
                    )
                for g in range(FD):
                    nc.tensor.matmul(
                        acc[:, :], stat[:, :, g], mov[:, :, g],
                        start=(c == 0 and g == 0),
                        stop=(c == CHUNKS - 1 and g == FD - 1),
                    )
            res = out_pool.tile([STAT_W, NB_L], mybir.dt.float32)
            nc.vector.tensor_copy(res[:], acc[:])
            nc.gpsimd.dma_start(y.ap(), res[:])
    _split_excess_waits(nc)
    return nc


def recover_hist(M):
    """M: [19, 32] fp32 (exact ints). Row 0 = l-marginals C, rows 1..18 are
    sign rows S_m = 2*G_m - C with G_m[lb] = #{x : x >= 32m, x&31 == lb}.
    Returns the per-core [600] histogram (int64)."""
    M = np.asarray(M).astype(np.int64)
    C = M[0]
    G = np.zeros((STAT_W + 1, NB_L), np.int64)
    G[0] = C
    for m in range(1, STAT_W):
        G[m] = (M[m] + C) // 2
    joint = G[:STAT_W] - G[1:STAT_W + 1]
    return joint.reshape(-1)[:MINLENGTH]


_NC_CACHE = {}


def get_nc():
    if "nc" not in _NC_CACHE:
        _NC_CACHE["nc"] = build_kernel()
    return _NC_CACHE["nc"]


def make_in_maps(x):
    x = np.ascontiguousarray(np.asarray(x, dtype=np.int32))
    assert x.shape == (N_TOTAL,), x.shape
    per = N_TOTAL // N_CORES
    return [
        {"x": x[c * per:(c + 1) * per].reshape(P, COLS)} for c in range(N_CORES)
    ]


def kernel(x):
    nc = get_nc()
    in_maps = make_in_maps(x)
    res = run_bass_kernel_spmd(nc, in_maps, core_ids=list(range(N_CORES)))
    hist = np.zeros(MINLENGTH, np.int64)
    for c in range(N_CORES):
        hist += recover_hist(res.results[c]["y"])
    return hist.astype(np.int32)


# revision 2
# speedup vs baseline: 15.8593x; 15.8593x over previous
"""Trainium2 Bass kernel: 600-bin bincount of 33.5M int32 values in [0, 600).

Strategy (data-parallel over 8 NeuronCores, per the sharding hint):
  - shard x into 8 slices of 4,194,304 elements, one per core, laid out
    [128 partitions, 32768] in DRAM;
  - per core, stream [128, FD] chunks into SBUF and compute, per element,
    a bilinear feature factorization of the bin index (600 = 19*32):
      * moving side  (VectorE):  one-hot of l = x & 31   (32 fp16 features)
      * stationary side (ScalarE): ones + Sign(x - 32m + 0.5), m=1..18
        (cumulative >= threshold features of h = x >> 5, as +-1 signs)
  - TensorE contracts each 128-element group: one matmul per group
    accumulating the 19x32 joint count matrix into a single PSUM tile
    (fp32 accumulate is exact: all entries < 2^24);
  - host recovers the exact joint histogram from the sign algebra and
    sums the 8 per-core histograms.
"""

from contextlib import ExitStack

import numpy as np

import bass_rust
import concourse.bass as bass
import concourse.mybir as mybir
import concourse.tile as tile
from concourse.bass_utils import run_bass_kernel_spmd

N_TOTAL = 33554432
N_CORES = 8
P = 128
COLS = N_TOTAL // N_CORES // P  # 32768 elements per partition per core
FD = 512                        # chunk free-dim
CHUNKS = COLS // FD             # 64
NB_L = 32                       # moving one-hot width (l = x & 31)
STAT_W = 19                     # ones + 18 sign thresholds (h = x >> 5 in [0,19))
MINLENGTH = 600


def _split_excess_waits(nc, max_waits=1):
    """This walrus build accepts at most one semaphore wait per instruction
    on several instruction structs; hoist excess waits onto preceding
    same-engine Drains (engines execute in order, so a chain of single-wait
    drains is equivalent to one multi-wait instruction)."""
    for f in nc.m.functions:
        for bb in f.blocks:
            out = []
            changed = False
            for ins in bb.instructions:
                si = ins.sync_info
                if si is not None and len(si.on_wait) > max_waits:
                    waits = list(si.on_wait)
                    chunks = [
                        waits[j : j + max_waits]
                        for j in range(0, len(waits), max_waits)
                    ]
                    for ci, chunk in enumerate(chunks[:-1]):
                        pre = mybir.InstDrain(
                            name=f"{ins.name}-presplit{ci}", ins=[], outs=[]
                        )
                        pre.engine = ins.engine
                        pre.sync_info = bass_rust.SyncInfo(
                            on_wait=chunk, on_update=[]
                        )
                        out.append(pre)
                        changed = True
                    ins.sync_info = bass_rust.SyncInfo(
                        on_wait=chunks[-1], on_update=list(si.on_update)
                    )
                out.append(ins)
            if changed:
                bb.instructions = out


def _reg_const(nc, val):
    val = float(val)
    if (mybir.dt.float32, val) in nc.const_aps.aps:
        return
    t = nc.alloc_sbuf_tensor(
        f"constf32_{abs(val)}_{'n' if val < 0 else 'p'}", [128, 1], mybir.dt.float32
    )
    nc.gpsimd.memset(t.ap(), val)
    nc.const_aps.aps[(mybir.dt.float32, val)] = t.ap()


def build_kernel(chunks=CHUNKS):
    nc = bass.Bass("TRN2", target_bir_lowering=False, debug=False)
    cols = chunks * FD
    x = nc.dram_tensor("x", [P, cols], mybir.dt.int32, kind="ExternalInput")
    y = nc.dram_tensor("y", [STAT_W, NB_L], mybir.dt.float32, kind="ExternalOutput")
    for m in range(1, STAT_W):
        _reg_const(nc, -(32.0 * m - 0.5))
    nc.all_engine_barrier()
    with tile.TileContext(nc) as tc:
        with tc.tile_pool(name="inp", bufs=2) as inp_pool, \
             tc.tile_pool(name="feat", bufs=2) as feat_pool, \
             tc.tile_pool(name="psum", bufs=1, space="PSUM") as psum_pool, \
             tc.tile_pool(name="outp", bufs=1) as out_pool:
            acc = psum_pool.tile([STAT_W, NB_L], mybir.dt.float32)
            for c in range(chunks):
                xi = inp_pool.tile([P, FD], mybir.dt.int32, tag="xi")
                nc.gpsimd.dma_start(xi[:], x.ap()[:, c * FD:(c + 1) * FD])
                x16 = feat_pool.tile([P, FD], mybir.dt.float16, tag="x16")
                nc.vector.tensor_copy(x16[:], xi[:])
                l32 = feat_pool.tile([P, FD], mybir.dt.int32, tag="l32")
                nc.vector.tensor_scalar(
                    l32[:], xi[:], 31, None, mybir.AluOpType.bitwise_and
                )
                l16 = feat_pool.tile([P, FD], mybir.dt.float16, tag="l16")
                nc.vector.tensor_copy(l16[:], l32[:])
                mov = feat_pool.tile([P, NB_L, FD], mybir.dt.float16, tag="mov")
                for lb in range(NB_L):
                    nc.vector.tensor_scalar(
                        mov[:, lb, :], l16[:], float(lb), None,
                        mybir.AluOpType.is_equal,
                    )
                stat = feat_pool.tile([P, STAT_W, FD], mybir.dt.float16, tag="stat")
                nc.gpsimd.memset(stat[:, 0, :], 1.0)
                for m in range(1, STAT_W):
                    nc.scalar.activation(
                        stat[:, m, :], x16[:],
                        mybir.ActivationFunctionType.Sign,
                        bias=-(32.0 * m - 0.5), scale=1.0,
                    )
                for g in range(FD):
                    nc.tensor.matmul(
                        acc[:, :], stat[:, :, g], mov[:, :, g],
                        start=(c == 0 and g == 0),
                        stop=(c == chunks - 1 and g == FD - 1),
                    )
            res = out_pool.tile([STAT_W, NB_L], mybir.dt.float32)
            nc.vector.tensor_copy(res[:], acc[:])
            nc.gpsimd.dma_start(y.ap(), res[:])
    _split_excess_waits(nc)
    return nc


def recover_hist(M):
    """M: [19, 32] fp32 (exact ints). Row 0 = l-marginals C, rows 1..18 are
    sign rows S_m = 2*G_m - C with G_m[lb] = #{x : x >= 32m, x&31 == lb}.
    Returns the per-core [600] histogram (int64)."""
    M = np.asarray(M).astype(np.int64)
    C = M[0]
    G = np.zeros((STAT_W + 1, NB_L), np.int64)
    G[0] = C
    for m in range(1, STAT_W):
        G[m] = (M[m] + C) // 2
    joint = G[:STAT_W] - G[1:STAT_W + 1]
    return joint.reshape(-1)[:MINLENGTH]


_NC_CACHE = {}


def get_nc():
    if "nc" not in _NC_CACHE:
        _NC_CACHE["nc"] = build_kernel()
    return _NC_CACHE["nc"]


def make_in_maps(x):
    x = np.ascontiguousarray(np.asarray(x, dtype=np.int32))
    assert x.shape == (N_TOTAL,), x.shape
    per = N_TOTAL // N_CORES
    return [
        {"x": x[c * per:(c + 1) * per].reshape(P, COLS)} for c in range(N_CORES)
    ]


def kernel(x):
    nc = get_nc()
    in_maps = make_in_maps(x)
    res = run_bass_kernel_spmd(nc, in_maps, core_ids=list(range(N_CORES)))
    hist = np.zeros(MINLENGTH, np.int64)
    for c in range(N_CORES):
        hist += recover_hist(res.results[c]["y"])
    return hist.astype(np.int32)


# revision 3
# speedup vs baseline: 16.5633x; 1.0444x over previous
"""Trainium2 Bass kernel: 600-bin bincount of 33.5M int32 values in [0, 600).

Strategy (data-parallel over 8 NeuronCores, per the sharding hint):
  - shard x into 8 slices of 4,194,304 elements, one per core, laid out
    [128 partitions, 32768] in DRAM;
  - per core, stream [128, FD] chunks into SBUF and compute, per element,
    a bilinear feature factorization of the bin index (600 = 19*32):
      * moving side  (VectorE):  one-hot of l = x & 31   (32 fp16 features)
      * stationary side (ScalarE): ones + Sign(x - 32m + 0.5), m=1..18
        (cumulative >= threshold features of h = x >> 5, as +-1 signs)
  - TensorE contracts each 128-element group: one matmul per group
    accumulating the 19x32 joint count matrix into a single PSUM tile
    (fp32 accumulate is exact: all entries < 2^24);
  - host recovers the exact joint histogram from the sign algebra and
    sums the 8 per-core histograms.
"""

from contextlib import ExitStack

import numpy as np

import bass_rust
import concourse.bass as bass
import concourse.mybir as mybir
import concourse.tile as tile
from concourse.bass_utils import run_bass_kernel_spmd

N_TOTAL = 33554432
N_CORES = 8
P = 128
COLS = N_TOTAL // N_CORES // P  # 32768 elements per partition per core
FD = 512                        # chunk free-dim
CHUNKS = COLS // FD             # 64
NB_L = 32                       # moving one-hot width (l = x & 31)
STAT_W = 19                     # ones + 18 sign thresholds (h = x >> 5 in [0,19))
MINLENGTH = 600


def _split_excess_waits(nc, max_waits=1):
    """This walrus build accepts at most one semaphore wait per instruction
    on several instruction structs; hoist excess waits onto preceding
    same-engine Drains (engines execute in order, so a chain of single-wait
    drains is equivalent to one multi-wait instruction)."""
    for f in nc.m.functions:
        for bb in f.blocks:
            out = []
            changed = False
            for ins in bb.instructions:
                si = ins.sync_info
                if si is not None and len(si.on_wait) > max_waits:
                    waits = list(si.on_wait)
                    chunks = [
                        waits[j : j + max_waits]
                        for j in range(0, len(waits), max_waits)
                    ]
                    for ci, chunk in enumerate(chunks[:-1]):
                        pre = mybir.InstDrain(
                            name=f"{ins.name}-presplit{ci}", ins=[], outs=[]
                        )
                        pre.engine = ins.engine
                        pre.sync_info = bass_rust.SyncInfo(
                            on_wait=chunk, on_update=[]
                        )
                        out.append(pre)
                        changed = True
                    ins.sync_info = bass_rust.SyncInfo(
                        on_wait=chunks[-1], on_update=list(si.on_update)
                    )
                out.append(ins)
            if changed:
                bb.instructions = out


def _reg_const(nc, val):
    val = float(val)
    if (mybir.dt.float32, val) in nc.const_aps.aps:
        return
    t = nc.alloc_sbuf_tensor(
        f"constf32_{abs(val)}_{'n' if val < 0 else 'p'}", [128, 1], mybir.dt.float32
    )
    nc.gpsimd.memset(t.ap(), val)
    nc.const_aps.aps[(mybir.dt.float32, val)] = t.ap()


def build_kernel(chunks=CHUNKS, repeat=1):
    nc = bass.Bass("TRN2", target_bir_lowering=False, debug=False)
    cols = chunks * FD
    x = nc.dram_tensor("x", [P, cols], mybir.dt.int32, kind="ExternalInput")
    y = nc.dram_tensor("y", [STAT_W, NB_L], mybir.dt.float32, kind="ExternalOutput")
    for m in range(1, STAT_W):
        _reg_const(nc, -(32.0 * m - 0.5))
    _reg_const(nc, 10000.0)
    nc.all_engine_barrier()
    total = chunks * repeat
    with tile.TileContext(nc) as tc:
        with tc.tile_pool(name="inp", bufs=2) as inp_pool, \
             tc.tile_pool(name="feat", bufs=3) as feat_pool, \
             tc.tile_pool(name="psum", bufs=1, space="PSUM") as psum_pool, \
             tc.tile_pool(name="outp", bufs=1) as out_pool:
            acc = psum_pool.tile([STAT_W, NB_L], mybir.dt.float32)
            for ci in range(total):
                c = ci % chunks
                xi = inp_pool.tile([P, FD], mybir.dt.int32, tag="xi")
                nc.gpsimd.dma_start(xi[:], x.ap()[:, c * FD:(c + 1) * FD])
                x16 = feat_pool.tile([P, FD], mybir.dt.float16, tag="x16")
                nc.vector.tensor_copy(x16[:], xi[:])
                l32 = feat_pool.tile([P, FD], mybir.dt.int32, tag="l32")
                nc.vector.tensor_scalar(
                    l32[:], xi[:], 31, None, mybir.AluOpType.bitwise_and
                )
                l16 = feat_pool.tile([P, FD], mybir.dt.float16, tag="l16")
                nc.vector.tensor_copy(l16[:], l32[:])
                mov = feat_pool.tile([P, NB_L, FD], mybir.dt.float16, tag="mov")
                for lb in range(NB_L):
                    nc.vector.tensor_scalar(
                        mov[:, lb, :], l16[:], float(lb), None,
                        mybir.AluOpType.is_equal,
                    )
                stat = feat_pool.tile([P, STAT_W, FD], mybir.dt.float16, tag="stat")
                nc.scalar.activation(
                    stat[:, 0, :], x16[:],
                    mybir.ActivationFunctionType.Sign,
                    bias=10000.0, scale=1.0,
                )
                for m in range(1, STAT_W):
                    nc.scalar.activation(
                        stat[:, m, :], x16[:],
                        mybir.ActivationFunctionType.Sign,
                        bias=-(32.0 * m - 0.5), scale=1.0,
                    )
                for g in range(FD):
                    nc.tensor.matmul(
                        acc[:, :], stat[:, :, g], mov[:, :, g],
                        start=(ci == 0 and g == 0),
                        stop=(ci == total - 1 and g == FD - 1),
                    )
            res = out_pool.tile([STAT_W, NB_L], mybir.dt.float32)
            nc.vector.tensor_copy(res[:], acc[:])
            nc.gpsimd.dma_start(y.ap(), res[:])
    _split_excess_waits(nc)
    return nc


def recover_hist(M):
    """M: [19, 32] fp32 (exact ints). Row 0 = l-marginals C, rows 1..18 are
    sign rows S_m = 2*G_m - C with G_m[lb] = #{x : x >= 32m, x&31 == lb}.
    Returns the per-core [600] histogram (int64)."""
    M = np.asarray(M).astype(np.int64)
    C = M[0]
    G = np.zeros((STAT_W + 1, NB_L), np.int64)
    G[0] = C
    for m in range(1, STAT_W):
        G[m] = (M[m] + C) // 2
    joint = G[:STAT_W] - G[1:STAT_W + 1]
    return joint.reshape(-1)[:MINLENGTH]


def build_kernel_rep(R=1):
    """R in-NEFF passes over the same input (device-timing harness)."""
    return build_kernel(repeat=R)


_NC_CACHE = {}


def get_nc():
    if "nc" not in _NC_CACHE:
        _NC_CACHE["nc"] = build_kernel()
    return _NC_CACHE["nc"]


def make_in_maps(x):
    x = np.ascontiguousarray(np.asarray(x, dtype=np.int32))
    assert x.shape == (N_TOTAL,), x.shape
    per = N_TOTAL // N_CORES
    return [
        {"x": x[c * per:(c + 1) * per].reshape(P, COLS)} for c in range(N_CORES)
    ]


def kernel(x):
    nc = get_nc()
    in_maps = make_in_maps(x)
    res = run_bass_kernel_spmd(nc, in_maps, core_ids=list(range(N_CORES)))
    hist = np.zeros(MINLENGTH, np.int64)
    for c in range(N_CORES):
        hist += recover_hist(res.results[c]["y"])
    return hist.astype(np.int32)


# revision 4
# speedup vs baseline: 32.5951x; 1.9679x over previous
"""Trainium2 Bass kernel: 600-bin bincount of 33.5M int32 values in [0, 600).

Strategy (data-parallel over 8 NeuronCores, per the sharding hint):
  - shard x into 8 slices of 4,194,304 elements, one per core, laid out
    [128 partitions, 8192, 4] in DRAM (4-group block-interleaving);
  - per core, stream chunks into SBUF and compute, per element, a bilinear
    feature factorization of the bin index (600 = 19*32):
      * moving side  (VectorE):  one-hot of l = x & 31  (32 fp16 features),
        stored block-interleaved [P, groups/4, 32, 4] so consecutive groups
        share SBUF lines on the TensorE moving-read path while DVE writes
        keep 8-byte-contiguous inner runs (fast perf modes);
      * stationary side (ScalarE): ones + Sign(x - 32m + 0.5), m=1..18
        (cumulative >= threshold features of h = x >> 5, as +-1 signs);
  - TensorE contracts each 128-element group: one matmul per group
    accumulating the 19x32 joint count matrix into a single PSUM tile
    (fp32 accumulate is exact: all entries < 2^24);
  - host recovers the exact joint histogram from the sign algebra and
    sums the 8 per-core histograms.

Measured ~740 us/core device time (vs ~47 us pure-DMA roofline); TensorE
instruction throughput (one matmul per 128-element group) is the bound.
"""

import numpy as np

import bass_rust
import concourse.bass as bass
import concourse.mybir as mybir
import concourse.tile as tile
from concourse.bass_utils import run_bass_kernel_spmd

N_TOTAL = 33554432
N_CORES = 8
P = 128
COLS = N_TOTAL // N_CORES // P  # 32768 elements per partition per core
FD = 512                        # groups per chunk
CHUNKS = COLS // FD             # 64
BLK = 4                         # group block-interleave factor
GPC = FD // BLK                 # group-blocks per chunk
NB_L = 32                       # moving one-hot width (l = x & 31)
STAT_W = 19                     # ones + 18 sign thresholds (h = x >> 5 in [0,19))
MINLENGTH = 600


def _split_excess_waits(nc, max_waits=1):
    """This walrus build accepts at most one semaphore wait per instruction
    on several instruction structs; hoist excess waits onto preceding
    same-engine Drains (engines execute in order, so a chain of single-wait
    drains is equivalent to one multi-wait instruction)."""
    for f in nc.m.functions:
        for bb in f.blocks:
            out = []
            changed = False
            for ins in bb.instructions:
                si = ins.sync_info
                if si is not None and len(si.on_wait) > max_waits:
                    waits = list(si.on_wait)
                    chunks = [
                        waits[j : j + max_waits]
                        for j in range(0, len(waits), max_waits)
                    ]
                    for ci, chunk in enumerate(chunks[:-1]):
                        pre = mybir.InstDrain(
                            name=f"{ins.name}-presplit{ci}", ins=[], outs=[]
                        )
                        pre.engine = ins.engine
                        pre.sync_info = bass_rust.SyncInfo(
                            on_wait=chunk, on_update=[]
                        )
                        out.append(pre)
                        changed = True
                    ins.sync_info = bass_rust.SyncInfo(
                        on_wait=chunks[-1], on_update=list(si.on_update)
                    )
                out.append(ins)
            if changed:
                bb.instructions = out


def _reg_const(nc, val):
    val = float(val)
    if (mybir.dt.float32, val) in nc.const_aps.aps:
        return
    t = nc.alloc_sbuf_tensor(
        f"constf32_{abs(val)}_{'n' if val < 0 else 'p'}", [128, 1], mybir.dt.float32
    )
    nc.gpsimd.memset(t.ap(), val)
    nc.const_aps.aps[(mybir.dt.float32, val)] = t.ap()


def build_kernel(chunks=CHUNKS, repeat=1):
    nc = bass.Bass("TRN2", target_bir_lowering=False, debug=False)
    x = nc.dram_tensor(
        "x", [P, chunks * GPC, BLK], mybir.dt.int32, kind="ExternalInput"
    )
    y = nc.dram_tensor("y", [STAT_W, NB_L], mybir.dt.float32, kind="ExternalOutput")
    for m in range(1, STAT_W):
        _reg_const(nc, -(32.0 * m - 0.5))
    _reg_const(nc, 10000.0)
    nc.all_engine_barrier()
    total = chunks * repeat
    with tile.TileContext(nc) as tc:
        with tc.tile_pool(name="inp", bufs=2) as inp_pool, \
             tc.tile_pool(name="feat", bufs=3) as feat_pool, \
             tc.tile_pool(name="psum", bufs=1, space="PSUM") as psum_pool, \
             tc.tile_pool(name="outp", bufs=1) as out_pool:
            acc = psum_pool.tile([STAT_W, NB_L], mybir.dt.float32)
            first = True
            for ci in range(total):
                c = ci % chunks
                xi = inp_pool.tile([P, GPC, BLK], mybir.dt.int32, tag="xi")
                nc.gpsimd.dma_start(xi[:], x.ap()[:, c * GPC:(c + 1) * GPC, :])
                x16 = feat_pool.tile([P, GPC, BLK], mybir.dt.float16, tag="x16")
                nc.vector.tensor_copy(x16[:], xi[:])
                l32 = feat_pool.tile([P, GPC, BLK], mybir.dt.int32, tag="l32")
                nc.vector.tensor_scalar(
                    l32[:], xi[:], 31, None, mybir.AluOpType.bitwise_and
                )
                l16 = feat_pool.tile([P, GPC, BLK], mybir.dt.float16, tag="l16")
                nc.vector.tensor_copy(l16[:], l32[:])
                mov = feat_pool.tile(
                    [P, GPC, NB_L, BLK], mybir.dt.float16, tag="mov"
                )
                for lb in range(NB_L):
                    nc.vector.tensor_scalar(
                        mov[:, :, lb, :], l16[:], float(lb), None,
                        mybir.AluOpType.is_equal,
                    )
                stat = feat_pool.tile(
                    [P, STAT_W, GPC, BLK], mybir.dt.float16, tag="stat"
                )
                nc.scalar.activation(
                    stat[:, 0, :, :], x16[:],
                    mybir.ActivationFunctionType.Sign,
                    bias=10000.0, scale=1.0,
                )
                for m in range(1, STAT_W):
                    nc.scalar.activation(
                        stat[:, m, :, :], x16[:],
                        mybir.ActivationFunctionType.Sign,
                        bias=-(32.0 * m - 0.5), scale=1.0,
                    )
                for gh in range(GPC):
                    for gl in range(BLK):
                        nc.tensor.matmul(
                            acc[:, :], stat[:, :, gh, gl], mov[:, gh, :, gl],
                            start=first,
                            stop=(ci == total - 1 and gh == GPC - 1
                                  and gl == BLK - 1),
                        )
                        first = False
            res = out_pool.tile([STAT_W, NB_L], mybir.dt.float32)
            nc.vector.tensor_copy(res[:], acc[:])
            nc.gpsimd.dma_start(y.ap(), res[:])
    _split_excess_waits(nc)
    return nc


def recover_hist(M):
    """M: [19, 32] fp32 (exact ints). Row 0 = l-marginals C, rows 1..18 are
    sign rows S_m = 2*G_m - C with G_m[lb] = #{x : x >= 32m, x&31 == lb}.
    Returns the per-core [600] histogram (int64)."""
    M = np.asarray(M).astype(np.int64)
    C = M[0]
    G = np.zeros((STAT_W + 1, NB_L), np.int64)
    G[0] = C
    for m in range(1, STAT_W):
        G[m] = (M[m] + C) // 2
    joint = G[:STAT_W] - G[1:STAT_W + 1]
    return joint.reshape(-1)[:MINLENGTH]


def build_kernel_rep(R=1):
    """R in-NEFF passes over the same input (device-timing harness)."""
    return build_kernel(repeat=R)


_NC_CACHE = {}


def get_nc():
    if "nc" not in _NC_CACHE:
        _NC_CACHE["nc"] = build_kernel()
    return _NC_CACHE["nc"]


def make_in_maps(x):
    x = np.ascontiguousarray(np.asarray(x, dtype=np.int32))
    assert x.shape == (N_TOTAL,), x.shape
    per = N_TOTAL // N_CORES
    return [
        {"x": x[c * per:(c + 1) * per].reshape(P, CHUNKS * GPC, BLK)}
        for c in range(N_CORES)
    ]


def kernel(x):
    nc = get_nc()
    in_maps = make_in_maps(x)
    res = run_bass_kernel_spmd(nc, in_maps, core_ids=list(range(N_CORES)))
    hist = np.zeros(MINLENGTH, np.int64)
    for c in range(N_CORES):
        hist += recover_hist(res.results[c]["y"])
    return hist.astype(np.int32)
